# revision 36
# baseline (speedup 1.0000x reference)
# GQA attention (RoPE, causal) for Trainium2, sharded over 8 NeuronCores.
#
# Reference semantics (B=2, T=2048, HIDDEN=2048, 16 q-heads, 4 kv-heads,
# head_dim=128, rotate-half RoPE, causal softmax, o-projection).
#
# Sharding: core c = (b, g) with b = c // 4 (batch), g = c % 4 (kv group).
# Each core computes q/k/v projections for its 4 q-heads + 1 kv head,
# attention, and a partial o-projection over its 512 columns of Wo; the
# host sums the 4 partials per batch.
#
# On-device layout is "transposed space": activations keep the feature
# dim on SBUF partitions and tokens on the free dim, so every matmul
# contraction (hidden, head_dim, seq) lands on the partition axis:
#   qT/kT:  [d=128, t]      (bf16; RoPE applied in this layout)
#   scoresT [s=128-chunk, t] = kT.T-chunk @ qT            (PE, bf16)
#   pT = exp(scale * scoresT)  in fp16                     (ACT; no max
#        pass: scaled scores are bounded ~7, fp16 range is plenty)
#   attnT   [d, t] += matmul(lhsT=v[s,d], rhs=pT[s,t])     (PE, fp16)
#   accT    [s, t] += pT                                   (DVE fp16 2x;
#        replaces the per-sig ones-matmul denominators -> ~30us less PE)
#   dnT = matmul(ones[128,128], accT)  once per (ti,head)  (PE bcast)
#   attnT_norm = attnT * recip(dnT)                        (DVE)
#   o[t, :] += attnT_norm.T-chunk @ WoT  (PE, bf16; partials DMA'd bf16)
#
# Projection scheduling: ti=0 is h-outer (6 PSUM accumulators chase the
# x-quarter DMA arrivals); ti=1..3 are j-sequential on a 2-bank rotation
# so each group's RoPE overlaps the next group's matmuls instead of
# piling up in one DVE burst. Attention PSUM tags land on the banks that
# ti=0 frees early, so attention overlaps the projection tail.
import os

import numpy as np

B, T, HIDDEN = 2, 2048, 2048
NH, NKV, D = 16, 4, 128
G = NH // NKV          # q-heads per kv group (4)
JQ = G * D             # q columns per group (512)
HC = HIDDEN // 128     # hidden chunks (16)
TC = T // 128          # token 128-chunks (16)
NT = T // 512          # token 512-tiles (4)
ROPE_THETA = 10000.0
SCALE = D ** -0.5

MODE = os.environ.get("ATTN_MM_MODE", "bf16")

_prog_cache = {}


def _np_io_dtype():
    import ml_dtypes

    return np.dtype(ml_dtypes.bfloat16)


def _build_program(reps=1):
    from contextlib import ExitStack

    import concourse.bass as bass
    import concourse.mybir as mybir
    import concourse.tile as tile
    from concourse import bacc
    from concourse.bass import ts

    dt = mybir.dt
    f32 = dt.float32
    bf16 = dt.bfloat16
    f16 = dt.float16

    Alu = mybir.AluOpType
    AF = mybir.ActivationFunctionType

    nc = bacc.Bacc(
        "TRN2", target_bir_lowering=False, debug=False, num_devices=8
    )

    xT_d = nc.dram_tensor("xT", [HIDDEN, T], bf16, kind="ExternalInput").ap()
    wqT_d = nc.dram_tensor("wqT", [HIDDEN, JQ], bf16, kind="ExternalInput").ap()
    wkT_d = nc.dram_tensor("wkT", [HIDDEN, D], bf16, kind="ExternalInput").ap()
    wvT_d = nc.dram_tensor("wvT", [HIDDEN, D], bf16, kind="ExternalInput").ap()
    woT_d = nc.dram_tensor("woT", [JQ, HIDDEN], bf16, kind="ExternalInput").ap()
    cos_d = nc.dram_tensor("cosC", [64, T], bf16, kind="ExternalInput").ap()
    sin_d = nc.dram_tensor("sinS", [64, T], bf16, kind="ExternalInput").ap()
    msk_d = nc.dram_tensor("cmask", [128, 4 * 512], f16, kind="ExternalInput").ap()
    idn_d = nc.dram_tensor("ident", [128, 128], f16, kind="ExternalInput").ap()
    o_d = nc.dram_tensor("o", [T, HIDDEN], bf16, kind="ExternalOutput").ap()

    xT_v = xT_d.rearrange("(hc p) t -> p hc t", p=128)     # [128, 16, T]
    wqT_v = wqT_d.rearrange("(hc p) j -> p hc j", p=128)   # [128, 16, 512]
    wkT_v = wkT_d.rearrange("(hc p) j -> p hc j", p=128)   # [128, 16, 128]
    wvT_v = wvT_d.rearrange("(hc p) j -> p hc j", p=128)   # [128, 16, 128]
    woT_v = woT_d.rearrange("(jc p) i -> p jc i", p=128)   # [128, 4, 2048]
    o_v = o_d.rearrange("(tc p) i -> p tc i", p=128)       # [128, 16, 2048]

    def rope(ptmp, src_ps, dst, cos_sb, sin_sb, tsl):
        # Stage PSUM->SBUF bf16 on ACT, then 5 bf16 DVE ops at 2x mode.
        # dst[0:64] = src*cos - src[64:]*sin ; dst[64:] = src*cos + src[:64]*sin
        # cos/sin rows are duplicated into both partition halves so every
        # SBUF+SBUF tensor_tensor reads partition-aligned operands (walrus
        # requires equal base partitions when both inputs are in SBUF).
        qsb = ptmp.tile([128, 512], bf16, name="rqsb")
        nc.scalar.copy(qsb, src_ps)
        tmp = ptmp.tile([128, 512], bf16, name="rtmp")
        qc = ptmp.tile([128, 512], bf16, name="rqc")
        nc.vector.tensor_tensor(tmp[0:64], qsb[64:128], sin_sb[64:128, tsl], Alu.mult)
        nc.vector.tensor_tensor(tmp[64:128], qsb[0:64], sin_sb[0:64, tsl], Alu.mult)
        nc.vector.tensor_tensor(qc, qsb, cos_sb[:, tsl], Alu.mult)
        nc.vector.tensor_tensor(dst[0:64], qc[0:64], tmp[0:64], Alu.subtract)
        nc.vector.tensor_tensor(dst[64:128], qc[64:128], tmp[64:128], Alu.add)

    with tile.TileContext(nc) as tc, ExitStack() as ctx:
      for _rep in range(reps):
        pers_cm = tc.tile_pool(name="pers", bufs=1)
        pers = pers_cm.__enter__()
        qr_sb = pers.tile([128, G, T], bf16, name="qr")
        kr_sb = pers.tile([128, T], bf16, name="kr")
        v_sb = pers.tile([128, TC, D], f16, name="vnat")
        mask_sb = pers.tile([128, 4 * 512], f16, name="cmask_sb")
        ones_sb = pers.tile([128, 128], f16, name="ones")
        cos_sb = pers.tile([128, T], bf16, name="cos")
        sin_sb = pers.tile([128, T], bf16, name="sin")
        ident = pers.tile([128, 128], f16, name="identsb")
        attnT_sb = pers.tile([128, G, T], bf16, name="attnT")
        woT_sb = pers.tile([128, G, HIDDEN], bf16, name="wo")

        with (
            tc.tile_pool(name="wpool", bufs=1) as wpool,
            tc.tile_pool(name="xpool", bufs=8) as xpool,
            tc.tile_pool(name="ptmp", bufs=3) as ptmp,
            tc.tile_pool(name="ntmp", bufs=3) as ntmp,
            tc.tile_pool(name="ptile", bufs=10) as ptile,
            tc.tile_pool(name="accp", bufs=3) as accp,
            tc.tile_pool(name="stg", bufs=4) as stg,
        ):
            # x quarters stream on the SP HWDGE ring; weights/trig/masks on
            # the ACT ring, so the first matmul's inputs arrive in parallel.
            wq_q, xtq = [], {}

            def load_xtq(ti, qtr):
                t_ = xpool.tile([128, 4, 512], bf16, name="xtq")
                nc.sync.dma_start(t_, xT_v[:, ts(qtr, 4), ts(ti, 512)])
                xtq[(ti, qtr)] = t_

            t00 = xpool.tile([128, 4, 512], bf16, name="xtq")
            nc.sync.dma_start(t00[:, 0:2], xT_v[:, 0:2, ts(0, 512)])
            nc.sync.dma_start(t00[:, 2:4], xT_v[:, 2:4, ts(0, 512)])
            xtq[(0, 0)] = t00
            wk_sb = wpool.tile([128, HC, D], bf16, name="wk")
            nc.scalar.dma_start(wk_sb, wkT_v)
            wv_sb = wpool.tile([128, HC, D], bf16, name="wv")
            nc.scalar.dma_start(wv_sb, wvT_v)
            wq_q.append(wpool.tile([128, 4, JQ], bf16, name="wqq0"))
            nc.scalar.dma_start(wq_q[0], wqT_v[:, ts(0, 4)])
            load_xtq(0, 1)
            wq_q.append(wpool.tile([128, 4, JQ], bf16, name="wqq1"))
            nc.scalar.dma_start(wq_q[1], wqT_v[:, ts(1, 4)])
            load_xtq(0, 2)
            wq_q.append(wpool.tile([128, 4, JQ], bf16, name="wqq2"))
            nc.scalar.dma_start(wq_q[2], wqT_v[:, ts(2, 4)])
            load_xtq(0, 3)
            wq_q.append(wpool.tile([128, 4, JQ], bf16, name="wqq3"))
            nc.scalar.dma_start(wq_q[3], wqT_v[:, ts(3, 4)])
            wq_t = [wq_q[h // 4][:, h % 4] for h in range(HC)]
            wk_t = [wk_sb[:, h] for h in range(HC)]
            wv_t = [wv_sb[:, h] for h in range(HC)]
            nc.scalar.dma_start(cos_sb[0:64], cos_d)
            nc.scalar.dma_start(sin_sb[0:64], sin_d)
            nc.vector.tensor_copy(cos_sb[64:128], cos_sb[0:64])
            nc.vector.tensor_copy(sin_sb[64:128], sin_sb[0:64])
            nc.scalar.dma_start(mask_sb, msk_d)
            nc.scalar.dma_start(ident, idn_d)
            nc.gpsimd.memset(ones_sb, 1.0)
            # warm the ACT exp spline tables (~2.7us load) during the
            # projection phase instead of at the first attention exp
            warm = ptmp.tile([1, 8], f16, name="actwarm")
            nc.scalar.activation(warm, ones_sb[0:1, 0:8], AF.Exp, scale=1.0)

            with tc.tile_pool(name="pp", bufs=1, space="PSUM") as pp:
                # ---------------- ti=0: h-outer, 6 single-buf accumulators
                ti = 0
                tsl = ts(ti, 512)
                for qtr in range(4):
                    load_xtq(1, qtr)
                xt = [xtq[(0, h // 4)][:, h % 4] for h in range(HC)]
                q_pss = [
                    pp.tile([128, 512], f32, name=f"q_ps{j}") for j in range(G)
                ]
                k_ps = pp.tile([128, 512], f32, name="k_ps")
                vt_ps = pp.tile([128, 512], f32, name="vt_ps")
                for h in range(HC):
                    st, sp = h == 0, h == HC - 1
                    nc.tensor.matmul(k_ps, wk_t[h], xt[h], start=st, stop=sp)
                    nc.tensor.matmul(vt_ps, wv_t[h], xt[h], start=st, stop=sp)
                    for j in range(G):
                        nc.tensor.matmul(
                            q_pss[j], wq_t[h][:, ts(j, 128)], xt[h],
                            start=st, stop=sp,
                        )
                # v epilogue first so its transposes/copies aren't stuck
                # behind the rope burst in the DVE queue
                vt_sb = ptmp.tile([128, 512], f16, name="vt_sb")
                nc.scalar.copy(vt_sb, vt_ps)
                for c in range(4):
                    v_ps = pp.tile([128, 128], f16, name="vt_ps")
                    nc.tensor.transpose(v_ps, vt_sb[:, ts(c, 128)], ident)
                    nc.vector.tensor_copy(v_sb[:, c, :], v_ps)
                for j in range(G):
                    rope(ptmp, q_pss[j], qr_sb[:, j, tsl], cos_sb, sin_sb, tsl)
                rope(ptmp, k_ps, kr_sb[:, tsl], cos_sb, sin_sb, tsl)

                # ---------------- ti=1..3: j-sequential groups, 2-bank rotation
                for ti in range(1, NT):
                    tsl = ts(ti, 512)
                    if ti + 1 < NT:
                        for qtr in range(4):
                            load_xtq(ti + 1, qtr)
                    xt = [xtq[(ti, h // 4)][:, h % 4] for h in range(HC)]
                    outs = [("q", 0), ("q", 1), ("q", 2), ("q", 3),
                            ("k", None), ("v", None)]
                    for kind, j in outs:
                        prj = pp.tile([128, 512], f32, name="prj", bufs=2)
                        for h in range(HC):
                            w_ap = {
                                "q": (lambda hh: wq_t[hh][:, ts(j, 128)]),
                                "k": (lambda hh: wk_t[hh]),
                                "v": (lambda hh: wv_t[hh]),
                            }[kind](h)
                            nc.tensor.matmul(
                                prj, w_ap, xt[h], start=(h == 0), stop=(h == HC - 1)
                            )
                        if kind == "q":
                            rope(ptmp, prj, qr_sb[:, j, tsl], cos_sb, sin_sb, tsl)
                        elif kind == "k":
                            rope(ptmp, prj, kr_sb[:, tsl], cos_sb, sin_sb, tsl)
                        else:
                            vt_sb = ptmp.tile([128, 512], f16, name="vt_sb")
                            nc.scalar.copy(vt_sb, prj)
                            for c in range(4):
                                # transposes rotate on ti=0's dead vt bank so
                                # they never hold up the 2-bank prj rotation
                                v_ps = pp.tile([128, 128], f16, name="vt_ps")
                                nc.tensor.transpose(v_ps, vt_sb[:, ts(c, 128)],
                                                    ident)
                                nc.vector.tensor_copy(v_sb[:, ti * 4 + c, :],
                                                      v_ps)

            # woT is DMA'd only now so the 2MB transfer never delays the
            # x-quarter stream; it lands well before the first o-proj weave.
            nc.scalar.dma_start(woT_sb, woT_v)

            # ---------------- attention + fused o-projection.
            # atp lands on ti=0's first two PSUM banks (freed early), scp on
            # the next six; the j-sequential prj banks are never reused, so
            # attention needs no sync against the projection tail.
            with (
                tc.tile_pool(name="atp", bufs=3, space="PSUM") as atp,
                tc.tile_pool(name="scp", bufs=2, space="PSUM") as scp,
                tc.tile_pool(name="dnp", bufs=1, space="PSUM") as dnp,
            ):
                def oproj_part(src_ti, c, ics, stage):
                    # partial o-projection chunks for one 128-token chunk;
                    # stage copies alternate ACT/DVE so neither queue chokes
                    tcx = 4 * src_ti + c
                    for ic in ics:
                        o_ps = atp.tile([128, 512], f32, name="at_ps")
                        for hj in range(G):
                            nc.tensor.matmul(
                                o_ps,
                                attnT_sb[:, hj, ts(tcx, 128)],
                                woT_sb[:, hj, ts(ic, 512)],
                                start=(hj == 0),
                                stop=(hj == G - 1),
                            )
                        if ic % 2 == 0:
                            nc.scalar.copy(stage[:, ts(ic, 512)], o_ps)
                        else:
                            nc.vector.tensor_copy(stage[:, ts(ic, 512)], o_ps)
                        if src_ti == NT - 1:
                            nc.sync.dma_start(
                                o_v[:, tcx, ts(ic, 512)], stage[:, ts(ic, 512)]
                            )
                    if src_ti < NT - 1 and ics[-1] == 3:
                        nc.sync.dma_start(o_v[:, tcx, :], stage)

                def oproj_tile(src_ti, c):
                    stage = stg.tile([128, HIDDEN], bf16, name="stage")
                    oproj_part(src_ti, c, [0, 1, 2, 3], stage)

                for ti in range(NT):
                    tsl = ts(ti, 512)
                    nblk = 4 * ti + 4
                    for hh in range(G):
                        at_ps = atp.tile([128, 512], f32, name="at_ps")
                        acc = accp.tile([128, 512], f16, name="acc")
                        # first third of the block chain accumulates on the
                        # otherwise-idle GPSIMD engine (full-width blocks
                        # only, done early, merged into acc at the end)
                        n_gp = nblk // 3 if nblk >= 8 else 0
                        acc_o = accp.tile([128, 512], f16, name="acco") \
                            if n_gp >= 2 else None

                        def emit_block(pt, base, sig, first, last):
                            # pt cols [base, base+512) hold block `sig` for
                            # this tile's t-window. Diagonal blocks r0>=1:
                            # columns t < 128*r0 are entirely above the
                            # causal line -- skipped by off.
                            r0 = sig - 4 * ti
                            off = max(0, r0) * 128
                            if r0 >= 0:
                                ms = 128 * r0
                                nc.vector.tensor_tensor(
                                    pt[:, base + ms : base + ms + 128],
                                    pt[:, base + ms : base + ms + 128],
                                    mask_sb[:, r0 * 512 + ms : r0 * 512 + ms + 128],
                                    Alu.mult,
                                )
                            nc.tensor.matmul(
                                at_ps[:, off:512],
                                v_sb[:, sig, :],
                                pt[:, base + off : base + 512],
                                start=first,
                                stop=last,
                            )
                            if acc_o is not None and sig < n_gp:
                                if first:
                                    nc.vector.tensor_copy(
                                        acc_o, pt[:, base : base + 512])
                                else:
                                    nc.gpsimd.tensor_tensor(
                                        acc_o, acc_o,
                                        pt[:, base : base + 512], Alu.add,
                                    )
                            elif first or (acc_o is not None and sig == n_gp):
                                nc.vector.tensor_copy(acc, pt[:, base : base + 512])
                            else:
                                nc.vector.tensor_tensor(
                                    acc[:, off:512],
                                    acc[:, off:512],
                                    pt[:, base + off : base + 512],
                                    Alu.add,
                                )

                        sig = 0
                        while sig < nblk:
                            # pack any two consecutive blocks into one 2-bank
                            # sc tile: halves the slot-rotation pressure that
                            # stalls each head boundary
                            pair = sig + 1 < nblk
                            sc = scp.tile([128, 1024], f32, name="sc")
                            sigs = [sig, sig + 1] if pair else [sig]
                            for k, sg in enumerate(sigs):
                                r0 = sg - 4 * ti
                                off = max(0, r0) * 128
                                nc.tensor.matmul(
                                    sc[:, 512 * k + off : 512 * (k + 1)],
                                    kr_sb[:, ts(sg, 128)],
                                    qr_sb[:, hh, bass.ds(512 * ti + off, 512 - off)],
                                    start=True,
                                    stop=True,
                                )
                            pt = ptile.tile([128, 1024], f16, name="pt")
                            if pair and sigs[1] - 4 * ti <= 0:
                                # both blocks full-width: one contiguous exp
                                nc.scalar.activation(pt, sc, AF.Exp, scale=SCALE)
                            else:
                                for k, sg in enumerate(sigs):
                                    r0 = sg - 4 * ti
                                    off = 512 * k + max(0, r0) * 128
                                    nc.scalar.activation(pt[:, off : 512 * (k + 1)],
                                                         sc[:, off : 512 * (k + 1)],
                                                         AF.Exp, scale=SCALE)
                            for k, sg in enumerate(sigs):
                                emit_block(pt, 512 * k, sg, sg == 0,
                                           sg == nblk - 1)
                            sig += 2 if pair else 1
                        # first half of the previous tile's o-projection:
                        # ready PE work that fills the ACT exp catch-up
                        # window at the head boundary
                        wstage = None
                        if ti > 0:
                            wstage = stg.tile([128, HIDDEN], bf16, name="stage")
                            oproj_part(ti - 1, hh, [0, 1], wstage)
                        if acc_o is not None:
                            nc.vector.tensor_tensor(acc, acc, acc_o, Alu.add)
                        # denominators: one partition-reduce+broadcast matmul
                        dn_ps = dnp.tile([128, 512], f32, name="dn")
                        nc.tensor.matmul(dn_ps, ones_sb, acc, start=True,
                                         stop=True)
                        rcp = ntmp.tile([128, 512], f32, name="rcp")
                        nc.vector.reciprocal_approx_fast(rcp, dn_ps)
                        nc.vector.tensor_tensor(
                            attnT_sb[:, hh, tsl], at_ps, rcp, Alu.mult
                        )
                        if ti > 0:
                            oproj_part(ti - 1, hh, [2, 3], wstage)
                for c in range(4):
                    oproj_tile(NT - 1, c)

        pers_cm.__exit__(None, None, None)

    nc.compile()
    return nc


def _host_shards(x, Wq, Wk, Wv, Wo):
    io_dt = _np_io_dtype()
    inv_freq = 1.0 / (
        ROPE_THETA ** (np.arange(0, D, 2, dtype=np.float32) / D)
    )  # [64]
    ang = np.arange(T, dtype=np.float32)[:, None] * inv_freq[None, :]  # [T, 64]
    cosC = np.ascontiguousarray(np.cos(ang).T.astype(io_dt))  # [64, T]
    sinS = np.ascontiguousarray(np.sin(ang).T.astype(io_dt))

    cmask = np.zeros((128, 4 * 512), np.float32)
    si = np.arange(128)[:, None]
    tj = np.arange(512)[None, :]
    for r in range(4):
        cmask[:, r * 512 : (r + 1) * 512] = (tj >= si + 128 * r).astype(np.float32)
    cmask = np.ascontiguousarray(cmask.astype(np.float16))

    ident = np.ascontiguousarray(np.eye(128, dtype=np.float16))

    in_maps = []
    for c in range(8):
        b, g = divmod(c, 4)
        in_maps.append(
            {
                "xT": np.ascontiguousarray(x[b].T).astype(io_dt),
                "wqT": np.ascontiguousarray(Wq[g * JQ : (g + 1) * JQ].T).astype(io_dt),
                "wkT": np.ascontiguousarray(Wk[g * D : (g + 1) * D].T).astype(io_dt),
                "wvT": np.ascontiguousarray(Wv[g * D : (g + 1) * D].T).astype(io_dt),
                "woT": np.ascontiguousarray(Wo[:, g * JQ : (g + 1) * JQ].T).astype(
                    io_dt
                ),
                "cosC": cosC,
                "sinS": sinS,
                "cmask": cmask,
                "ident": ident,
            }
        )
    return in_maps


def _run(x, Wq, Wk, Wv, Wo, trace=False, trace_kwargs=None):
    from concourse.bass_utils import run_bass_kernel_spmd

    if MODE not in _prog_cache:
        _prog_cache[MODE] = _build_program()
    nc = _prog_cache[MODE]
    in_maps = _host_shards(x, Wq, Wk, Wv, Wo)
    res = run_bass_kernel_spmd(
        nc, in_maps, core_ids=list(range(8)), trace=trace, **(trace_kwargs or {})
    )
    outs = [r["o"].astype(np.float32) for r in res.results]
    out = np.empty((B, T, HIDDEN), np.float32)
    for b in range(B):
        out[b] = outs[4 * b] + outs[4 * b + 1] + outs[4 * b + 2] + outs[4 * b + 3]
    return out, res


def kernel(x, mask, Wq, Wk, Wv, Wo):
    x = np.asarray(x, np.float32)
    Wq = np.asarray(Wq, np.float32)
    Wk = np.asarray(Wk, np.float32)
    Wv = np.asarray(Wv, np.float32)
    Wo = np.asarray(Wo, np.float32)
    out, _ = _run(x, Wq, Wk, Wv, Wo)
    return out
